# revision 25
# baseline (speedup 1.0000x reference)
"""DeltaNet attention TRN2 kernel (nn_DeltaNetAttention_5299989643476).

Strategy: data-parallel over batch (8 batches -> 8 NeuronCores). The
cross-batch cumulative_state scan is tiny ([H, Dh]) and is computed on the
host via an algebraic shortcut (mean over (b,l) of kv == Ksum . V
contraction), then passed to every core as a small constant tensor, so the
device program needs no collectives.

On-device, everything runs in a "transposed" layout (features on
partitions, sequence on the free dim):
  - QT/KT/VT projections: weight-stationary **fp8e4 DoubleRow** matmuls
    (two k-planes per instruction), fp32 PSUM accum. Weights are
    pre-scaled by a power of two on the host so their std is ~1 in fp8;
    the inverse scale is folded into the PSUM->SBUF copy. K/V land
    directly in fp8 (their consumers are fp8 matmuls); Q stays bf16 for
    the q-mod STT.
  - phi(x)=elu(x)+1 materialized as ONE fp8 operand per side:
    pk = max(K,0)+exp(min(K,0)) via min (DVE), Exp (ACT), then a single
    STT (max 0 then add) on DVE. The A matmul is then a single DoubleRow
    fp8 matmul per 128-key block (vs 4-way over separate phi halves).
  - per head: kvT matmul fp8 DoubleRow; q-mod via STT with per-partition
    cs; causal linear attention in fp8 with a 0.125-scaled mask (keeps am
    inside e4m3 range; the num/den ratio is scale-invariant); V comes
    back sequence-major through PE transposes.
  - output projection is interleaved INTO the per-head loop: row-block 0
    accumulates its 4 PSUM-bank quarters across heads (N=512 DoubleRow,
    weights moving), so the PE's head-phase bubbles are filled and only
    row-block 1's projection trails the last head. LayerNorm drains in
    fine chunks split across DVE and ACT, overlapping the output DMA.
"""

import numpy as np
import ml_dtypes

import concourse.bass as bass
import concourse.mybir as mybir
import concourse.tile as tile
from concourse import bacc
from concourse.bass_utils import run_bass_kernel_spmd
from concourse.masks import make_identity


def _ensure_axon_hooks():
    """This image's `antenv` lacks `axon_hooks`; if the caller's environment
    sets BASS_TRACE, run_bass_kernel_spmd would crash importing it. Register
    a no-op shim (only when absent) so tracing degrades gracefully."""
    try:
        import antenv.axon_hooks  # noqa: F401
    except ImportError:
        import sys
        import types

        import antenv

        mod = types.ModuleType("antenv.axon_hooks")
        _h = [None]
        mod.set_axon_ntff_profile_hook = lambda h: _h.__setitem__(0, h)
        mod.get_axon_ntff_profile_hook = lambda: _h[0]
        sys.modules["antenv.axon_hooks"] = mod
        antenv.axon_hooks = mod


_ensure_axon_hooks()

B, L, D, H = 8, 256, 2048, 8
DH = D // H            # 256
NB = D // 128          # 16 feature blocks of 128
LB = L // 128          # 2 sequence blocks of 128
EPS = 1e-5
MSCALE = 0.125         # causal-mask scale keeping am inside e4m3 range
NWARM = 80             # PE clock-ramp matmuls bridging the first panel DMA

F32 = mybir.dt.float32
BF16 = mybir.dt.bfloat16
FP8 = mybir.dt.float8e4
AF = mybir.ActivationFunctionType
OP = mybir.AluOpType
DR = mybir.MatmulPerfMode.DoubleRow

_cache = {}


def _build(alpha: float, invs: tuple, plain_ln: bool = False):
    nc = bacc.Bacc(
        "TRN2",
        target_bir_lowering=False,
        debug=False,
        enable_asserts=False,
        num_devices=B,
    )

    # inputs/weights arrive pre-panelized from the host so every DMA is a
    # contiguous >=2KB-per-partition block copy (runs <512B pay a 2x DMA
    # latency penalty)
    qT_d = nc.dram_tensor("qT", [128, NB, L], FP8, kind="ExternalInput")
    kT_d = nc.dram_tensor("kT", [128, NB, L], FP8, kind="ExternalInput")
    vT_d = nc.dram_tensor("vT", [128, NB, L], FP8, kind="ExternalInput")
    qres_d = nc.dram_tensor("qres", [L, D], BF16, kind="ExternalInput")
    wqT_d = nc.dram_tensor("wqT", [4, 128, NB, 512], FP8, kind="ExternalInput")
    wkT_d = nc.dram_tensor("wkT", [4, 128, NB, 512], FP8, kind="ExternalInput")
    wvT_d = nc.dram_tensor("wvT", [4, 128, NB, 512], FP8, kind="ExternalInput")
    woT_d = nc.dram_tensor("woT", [4, 128, NB, 512], FP8, kind="ExternalInput")
    csp_d = nc.dram_tensor("csp", [128, H * 2], F32, kind="ExternalInput")
    maskT_d = nc.dram_tensor("maskT", [L, L], BF16, kind="ExternalInput")
    lng_d = nc.dram_tensor("lng", [D], F32, kind="ExternalInput")
    lnb_d = nc.dram_tensor("lnb", [D], F32, kind="ExternalInput")
    out_d = nc.dram_tensor("out", [L, D], F32, kind="ExternalOutput")

    with tile.TileContext(nc) as tc:
        _body(
            tc, alpha, invs,
            qT_d, kT_d, vT_d, qres_d,
            wqT_d, wkT_d, wvT_d, woT_d,
            csp_d, maskT_d, lng_d, lnb_d, out_d,
            plain_ln,
        )
    nc.compile()
    return nc


def _body(tc, alpha, invs, qT_d, kT_d, vT_d, qres_d, wqT_d, wkT_d, wvT_d,
          woT_d, csp_d, maskT_d, lng_d, lnb_d, out_d, plain_ln):
    nc = tc.nc
    inv_k, inv_v, inv_q, inv_o = invs

    with (
        tc.tile_pool(name="singles", bufs=1) as singles,
        tc.tile_pool(name="wpool", bufs=6) as wpool,
        tc.tile_pool(name="wopool", bufs=4) as wopool,
        tc.tile_pool(name="big", bufs=1) as big,
        tc.tile_pool(name="hgrp", bufs=2) as hgrp,
        tc.tile_pool(name="small", bufs=3) as small,
        tc.tile_pool(name="ps", bufs=4, space="PSUM") as ps,
        tc.tile_pool(name="ops", bufs=1, space="PSUM") as ops,
    ):
        # ---- kernel-start DMA priority: the first real matmul chain needs
        # kT plus the whole first wk super-panel; fan them over five queues
        # so they land as early as possible.
        # fine-grained quarters so the first projection matmuls gate on
        # 128-256KB transfers instead of full-panel ones (subtile deps are
        # per-DMA): j2-pair p needs only kT[2p:2p+2] and wk0[2p:2p+2].
        # Everything start-critical rides sync/scalar (HWDGE): gpsimd DMAs
        # go through the much slower software-DGE descriptor path.
        w_rs = {"k": wkT_d, "v": wvT_d, "q": wqT_d, "o": woT_d}
        wk0_t = wpool.tile([128, NB, 512], FP8, tag="w", name="w_k0")
        wk0_r = w_rs["k"].ap()[0]

        xT_in = {}
        for name, dram in (("k", kT_d), ("v", vT_d), ("q", qT_d)):
            t = big.tile([128, NB, L], FP8, tag=f"{name}T_in", name=f"{name}T_in")
            xT_in[name] = (t, dram)

        kt_t, kt_r = xT_in["k"][0], kT_d.ap()
        nc.sync.dma_start(out=kt_t[:, 0:4, :], in_=kt_r[:, 0:4, :])
        nc.scalar.dma_start(out=wk0_t[:, 0:4, :], in_=wk0_r[:, 0:4, :])
        nc.sync.dma_start(out=wk0_t[:, 4:8, :], in_=wk0_r[:, 4:8, :])
        nc.scalar.dma_start(out=kt_t[:, 4:8, :], in_=kt_r[:, 4:8, :])
        nc.sync.dma_start(out=kt_t[:, 8:12, :], in_=kt_r[:, 8:12, :])
        nc.scalar.dma_start(out=wk0_t[:, 8:12, :], in_=wk0_r[:, 8:12, :])
        nc.sync.dma_start(out=wk0_t[:, 12:16, :], in_=wk0_r[:, 12:16, :])
        nc.scalar.dma_start(out=kt_t[:, 12:16, :], in_=kt_r[:, 12:16, :])

        def load_xT(name):
            # v/q inputs ride the otherwise-idle gpsimd (SWDGE) queue with a
            # ~15us deadline; keeps sync/scalar clear for the panel stream
            t, dram = xT_in[name]
            r = dram.ap()
            nc.gpsimd.dma_start(out=t[:, 0:8, :], in_=r[:, 0:8, :])
            nc.gpsimd.dma_start(out=t[:, 8:16, :], in_=r[:, 8:16, :])

        # out-proj weight panels: staggered through the k/v/q phases so the
        # 4MB doesn't pile onto the q-phase HBM window (where wq + qres
        # already saturate); own pool so the "w" tag rotation never makes a
        # later panel DMA wait on far-future out-proj readers
        wo_panels = []

        def load_wo(nq):
            w_t = wopool.tile([128, NB, 512], FP8, tag="wo", name=f"w_o{nq}")
            nc.gpsimd.dma_start(out=w_t, in_=w_rs["o"].ap()[nq])
            wo_panels.append(w_t)

        ones_t = singles.tile([128, 128], BF16)
        nc.vector.memset(ones_t, 1.0)
        ones8_t = singles.tile([128, 2, 128], FP8)
        nc.vector.memset(ones8_t, 1.0)
        eps_t = singles.tile([128, 1], F32)
        nc.vector.memset(eps_t, EPS)
        ident8 = singles.tile([128, 128], FP8)
        make_identity(nc, ident8)
        csp_t = singles.tile([128, H * 2], F32)
        nc.gpsimd.dma_start(out=csp_t, in_=csp_d.ap())
        maskT_t = singles.tile([128, LB, L], BF16)
        nc.gpsimd.dma_start(out=maskT_t,
                            in_=maskT_d.rearrange("(a p) l -> p a l", p=128))

        warm_exp = singles.tile([128, 1], F32)

        # dummy matmuls while the first weight panels stream in: continuous
        # PE work finishes the clock ramp to 2.4 GHz before the real stream
        # starts, and must BRIDGE the DMA wait (an idle gap resets the ramp)
        warm_ps = ps.tile([128, 256], F32, tag="ps", name="warm_ps")
        for _ in range(NWARM):
            nc.tensor.matmul(warm_ps[:, 0:128], ones_t, ones_t,
                             start=True, stop=True)

        succ = {"k": "v", "v": "q", "q": "o"}

        def panel_dma(name, sp):
            # 512-wide super-panels, each split into two j-half DMAs that
            # ride sync and scalar IN PARALLEL: the panel lands in half the
            # time, so the PE never outruns the weight stream
            if name == "k" and sp == 0:
                return wk0_t
            w_t = wpool.tile([128, NB, 512], FP8, tag="w", name=f"w_{name}{sp}")
            w_r = w_rs[name].ap()[sp]
            # q panels avoid the scalar queue: ACT is busy with psv copies
            # and Exp there, which would delay the DMA trigger itself
            eng2 = nc.sync if name == "q" else nc.scalar
            nc.sync.dma_start(out=w_t[:, 0:8, :], in_=w_r[:, 0:8, :])
            eng2.dma_start(out=w_t[:, 8:16, :], in_=w_r[:, 8:16, :])
            return w_t

        # ---- projections: XT[i, l] = sum_j WT[j, i] * xT[j, l] ----
        inv_x = {"k": inv_k, "v": inv_v, "q": inv_q}
        out_dt = {"k": FP8, "v": FP8, "q": BF16}
        projs = {}
        V_t = big.tile([128, LB, D], FP8, tag="V")
        pq8_t = big.tile([128, NB, L], FP8, tag="pq8")
        kvm_cur = [None]

        def emit_kv_head(h, QT_t):
            # kv matmul + q-mod STT + V transposes for head h; interleaved
            # into the q projection right after QT's i-quarter h lands, so
            # the PE fills q-panel stalls and DVE group work starts early
            n0 = 2 * h
            KT_t, VT_t = projs["k"], projs["v"]
            if h % 2 == 0:
                kvm_cur[0] = hgrp.tile([128, 4, L], BF16, tag="kvm",
                                       name=f"kvm{h//2}")
            kvm = kvm_cur[0]
            hh = h % 2
            kvp = ps.tile([128, 2, L], F32, tag="ps", name=f"kvp{h}")
            for mb in range(2):
                # kv DoubleRow fp8: both d-blocks of the head in one matmul
                nc.tensor.matmul(
                    kvp[:, mb, :],
                    VT_t[:, n0:n0 + 2, mb * 128:(mb + 1) * 128],
                    KT_t[:, n0:n0 + 2, :],
                    start=True, stop=True, perf_mode=DR,
                )
            for mb in range(2):
                # q_mod = (alpha*Q) * (kv + cs*(1-alpha)/alpha); alpha is
                # folded into Wq on the host, so one STT straight from PSUM
                # does modulate+multiply.
                nc.vector.scalar_tensor_tensor(
                    out=kvm[:, 2 * hh + mb, :],
                    in0=kvp[:, mb, :],
                    scalar=csp_t[:, n0 + mb:n0 + mb + 1],
                    in1=QT_t[:, n0 + mb, :],
                    op0=OP.add,
                    op1=OP.mult,
                )
            for ib in range(LB):
                # fp8 transpose writes PSUM at element step 2 (hw quirk);
                # lay psv out [row, 2] and gather the even bytes in the copy
                psv = ps.tile([128, 256, 2], FP8, tag="ps", name=f"psv{h}_{ib}")
                for db in range(2):
                    nc.tensor.transpose(
                        psv[:, db * 128:(db + 1) * 128, 0:1],
                        VT_t[:, n0 + db, ib * 128:(ib + 1) * 128],
                        ident8,
                    )
                nc.scalar.copy(
                    out=V_t[:, ib, h * DH:h * DH + 256], in_=psv[:, :, 0]
                )
            if h % 2 == 1:
                # phi(q_mod) for the finished group: one fused fp8 operand
                # pq = max(q,0) + exp(min(q,0)) via min, Exp, then STT
                g = h // 2
                qsl = slice(4 * g, 4 * g + 4)
                tn = hgrp.tile([128, 4, L], BF16, tag="tn")
                eQg = hgrp.tile([128, 4, L], FP8, tag="eq")
                nc.vector.tensor_scalar_min(tn, kvm, 0.0)
                nc.scalar.activation(eQg, tn, AF.Exp)
                nc.vector.scalar_tensor_tensor(
                    out=pq8_t[:, qsl, :], in0=kvm, scalar=0.0, in1=eQg,
                    op0=OP.max, op1=OP.add,
                )

        pk8_t = big.tile([128, NB, L], FP8, tag="pk8")
        eK_t = big.tile([128, NB, L], FP8, tag="eK")
        for name in ("k", "v", "q"):
            out_t = big.tile([128, NB, L], out_dt[name], tag=f"{name}proj",
                             name=f"{name}proj")
            x_t = xT_in[name][0]
            for sp in range(4):  # super-panel: 4 output feature blocks
                w_t = panel_dma(name, sp)
                if sp == 2 and succ[name] != "o":
                    load_xT(succ[name])
                if (name, sp) in (("k", 3), ("v", 1), ("v", 3), ("q", 1)):
                    load_wo(len(wo_panels))
                for half in range(2):
                    iq = 2 * sp + half
                    psp = ps.tile([128, 2, L], F32, tag="ps", name=f"pp{name}{iq}")
                    for ib in range(2):
                        ic = half * 2 + ib
                        for j2 in range(8):  # 128-block pairs: K=256/matmul
                            nc.tensor.matmul(
                                psp[:, ib, :],
                                w_t[:, 2 * j2:2 * j2 + 2,
                                    ic * 128:(ic + 1) * 128],
                                x_t[:, 2 * j2:2 * j2 + 2, :],
                                start=(j2 == 0),
                                stop=(j2 == 7),
                                perf_mode=DR,
                            )
                    # PSUM->SBUF copy folds in the fp8 weight-scale inverse
                    # (DVE only: a copy on ACT delays the odd panel DMAs)
                    nc.vector.tensor_scalar_mul(
                        out_t[:, iq * 2:iq * 2 + 2, :], psp, inv_x[name])
                    if name == "q":
                        emit_kv_head(iq, out_t)
            projs[name] = out_t
            if name == "k":
                # pre-load the Exp ACT table now: after the k panel DMAs on
                # the scalar queue, well before the first real Exp user
                nc.scalar.activation(warm_exp, eps_t, AF.Exp)
                # phi(K) exp half on DVE/ACT while the V projection streams:
                # eK = exp(min(K,0)), computed from the fp8 K
                nc.vector.tensor_scalar_min(eK_t, out_t, 0.0)
            if name == "v":
                nc.scalar.activation(eK_t, eK_t, AF.Exp)
                # fused phi(K) fp8 operand: pk = max(K,0) + eK
                nc.vector.scalar_tensor_tensor(
                    out=pk8_t, in0=projs["k"], scalar=0.0, in1=eK_t,
                    op0=OP.max, op1=OP.add,
                )
        KT_t, VT_t, QT_t = projs["k"], projs["v"], projs["q"]

        qres_t = []
        for lb in range(LB):
            t = big.tile([128, D], BF16, tag=f"qres{lb}", name=f"qres{lb}")
            nc.sync.dma_start(out=t, in_=qres_d.ap()[lb * 128:(lb + 1) * 128, :])
            qres_t.append(t)
        lng_t = lnb_t = None
        if not plain_ln:
            lng_t = singles.tile([128, D], F32)
            nc.gpsimd.dma_start(out=lng_t,
                                in_=lng_d.ap().partition_broadcast(128))
            lnb_t = singles.tile([128, D], F32)
            nc.gpsimd.dma_start(out=lnb_t,
                                in_=lnb_d.ap().partition_broadcast(128))

        # Trigger the sqrt ACT-table load now — after ScalarE's last Exp
        # user, off the LN critical path (the set switch costs ~2.6us).
        warm_sqrt = singles.tile([128, 1], F32)
        nc.scalar.activation(warm_sqrt, eps_t, AF.Sqrt)

        # ---- per-head: A matmul (fused phi operands), mask, den, num,
        # plus row-block-0 out-proj accumulation interleaved across heads ----
        attnT_t = big.tile([128, NB, L], FP8, tag="attnT")
        outp = ops.tile([128, 4, 512], F32, tag="op", name="outp")

        def emit_A(h):
            n0 = 2 * h
            a_ps = ps.tile([128, 2, L], F32, tag="ps", name=f"a{h}")
            nc.tensor.matmul(
                a_ps[:, 0, :],
                pk8_t[:, n0:n0 + 2, 0:128],
                pq8_t[:, n0:n0 + 2, :],
                start=True, stop=True, perf_mode=DR,
            )
            nc.tensor.matmul(
                a_ps[:, 1, 128:L],
                pk8_t[:, n0:n0 + 2, 128:L],
                pq8_t[:, n0:n0 + 2, 128:L],
                start=True, stop=True, perf_mode=DR,
            )
            return a_ps

        def emit_am_den(h, a_ps):
            # am = A * mask * MSCALE, in fp8 (mask carries the MSCALE).
            # DVE is the head-phase bottleneck: the mid (unmasked) block and
            # one triangular block route through ACT (identity / copy), the
            # copied triangle multiplies on the otherwise-idle GpSimd, the
            # other triangle stays on DVE straight from PSUM.
            am = small.tile([128, LB, L], FP8, tag="am")
            a_sb = small.tile([128, 2, 128], BF16, tag="asb")
            nc.scalar.copy(out=a_sb[:, 0, :], in_=a_ps[:, 0, 0:128])
            nc.scalar.activation(out=am[:, 0, 128:L], in_=a_ps[:, 0, 128:L],
                                 func=AF.Identity, scale=MSCALE)
            nc.scalar.copy(out=a_sb[:, 1, :], in_=a_ps[:, 1, 128:L])
            nc.gpsimd.tensor_mul(am[:, 0, 0:128], a_sb[:, 0, :],
                                 maskT_t[:, 0, 0:128])
            nc.gpsimd.tensor_mul(am[:, 1, 128:L], a_sb[:, 1, :],
                                 maskT_t[:, 1, 128:L])

            den_ps = ps.tile([128, L], F32, tag="ps", name=f"den{h}")
            nc.tensor.matmul(den_ps[:, 0:128], ones8_t[:, 0, :],
                             am[:, 0, 0:128], start=True, stop=True)
            nc.tensor.matmul(den_ps[:, 128:L], ones8_t,
                             am[:, 0:2, 128:L], start=True, stop=True,
                             perf_mode=DR)
            # den is a sum of strictly positive phi-products, so the
            # reference's 1e-8 clamp can never bind — reciprocal straight
            # from PSUM. MSCALE cancels between num and den.
            rden = small.tile([128, L], F32, tag="rden")
            nc.vector.reciprocal_approx_fast(out=rden, in_=den_ps)
            return am, rden

        def emit_num(h, am, rden):
            n0 = 2 * h
            n_ps = ps.tile([128, 2, L], F32, tag="ps", name=f"n{h}")
            for db in range(2):
                dsl = slice(h * DH + db * 128, h * DH + (db + 1) * 128)
                nc.tensor.matmul(n_ps[:, db, 0:128], V_t[:, 0, dsl],
                                 am[:, 0, 0:128], start=True, stop=True)
                nc.tensor.matmul(n_ps[:, db, 128:L], V_t[:, 0:2, dsl],
                                 am[:, 0:2, 128:L], start=True, stop=True,
                                 perf_mode=DR)
            for db in range(2):
                nc.vector.tensor_mul(attnT_t[:, n0 + db, :], n_ps[:, db, :],
                                     rden)

        def emit_oproj0(h):
            # head h's contribution to row-block 0 of the output projection:
            # one N=512 DoubleRow matmul per output quarter, accumulating in
            # 4 persistent PSUM banks across all heads
            for nq in range(4):
                nc.tensor.matmul(
                    outp[:, nq, :],
                    attnT_t[:, 2 * h:2 * h + 2, 0:128],
                    wo_panels[nq][:, 2 * h:2 * h + 2, :],
                    start=(h == 0),
                    stop=(h == H - 1),
                    perf_mode=DR,
                )

        # three-stage software pipeline: the PSUM->ACT->GpSimd->PE->DVE
        # chain per head is ~2.5us of latency, so each stage gets a full
        # head-slot of slack before its consumer runs
        stage = {}
        for i in range(H + 2):
            if i < H:
                stage[i] = [emit_A(i)]
            if 1 <= i <= H:
                h = i - 1
                stage[h] += emit_am_den(h, stage[h][0])
            if i >= 2:
                h = i - 2
                _, am, rden = stage.pop(h)
                emit_num(h, am, rden)
                emit_oproj0(h)

        # ---- residual + LayerNorm + out DMA ----
        # x is held in bf16: the drain STT / bn_stats / normalize ops on DVE
        # are SBUF-port-bound, so halving the element size nearly halves the
        # epilogue's DVE time. Final normalize still writes f32 output.
        x_sb = [big.tile([128, D], BF16, tag=f"x{lb}", name=f"x{lb}")
                for lb in range(LB)]
        xout = [big.tile([128, D], F32, tag=f"xo{lb}", name=f"xo{lb}")
                for lb in range(LB)]
        stats = [small.tile([128, 4, 6], F32, tag=f"stats{lb}",
                            name=f"stats{lb}", bufs=1) for lb in range(LB)]
        oengs = (nc.sync, nc.gpsimd, nc.scalar, nc.sync,
                 nc.gpsimd, nc.scalar, nc.sync, nc.gpsimd)

        def drain_quarter(lb, nq, src):
            # x = inv_o*o + (query + bo), running LN stats per quarter
            sl = slice(nq * 512, (nq + 1) * 512)
            nc.vector.scalar_tensor_tensor(
                out=x_sb[lb][:, sl], in0=src, scalar=inv_o,
                in1=qres_t[lb][:, sl], op0=OP.mult, op1=OP.add)
            nc.vector.bn_stats(out=stats[lb][:, nq, :], in_=x_sb[lb][:, sl])

        def emit_ln(lb):
            x = x_sb[lb]
            mv = small.tile([128, 2], F32, tag="mv")
            nc.vector.bn_aggr(out=mv, in_=stats[lb])
            sd = small.tile([128, 1], F32, tag="sd")
            nc.scalar.activation(sd, mv[:, 1:2], AF.Sqrt, bias=eps_t)
            nc.vector.reciprocal_approx_fast(out=sd, in_=sd)
            nsdmu = small.tile([128, 1], F32, tag="nsdmu")
            nc.vector.tensor_scalar(
                out=nsdmu, in0=sd, scalar1=mv[:, 0:1], scalar2=-1.0,
                op0=OP.mult, op1=OP.mult,
            )
            xo = xout[lb]
            if plain_ln:
                # ln_g == 1, ln_b == 0: fused (x - mu) * rstd in 8 chunks
                # split across DVE and the idle ScalarE (as rstd*x - rstd*mu)
                for ch in range(8):
                    sl = slice(ch * (D // 8), (ch + 1) * (D // 8))
                    if ch % 2 == 0:
                        nc.vector.tensor_scalar(
                            out=xo[:, sl], in0=x[:, sl], scalar1=mv[:, 0:1],
                            scalar2=sd, op0=OP.subtract, op1=OP.mult,
                        )
                    else:
                        nc.scalar.activation(
                            out=xo[:, sl], in_=x[:, sl], func=AF.Identity,
                            bias=nsdmu, scale=sd,
                        )
                    oengs[ch].dma_start(
                        out=out_d.ap()[lb * 128:(lb + 1) * 128, sl],
                        in_=xo[:, sl])
            else:
                for ch in range(4):
                    sl = slice(ch * (D // 4), (ch + 1) * (D // 4))
                    nc.vector.tensor_scalar(
                        out=xo[:, sl], in0=x[:, sl], scalar1=mv[:, 0:1],
                        scalar2=None, op0=OP.subtract,
                    )
                    nc.vector.scalar_tensor_tensor(
                        out=xo[:, sl], in0=xo[:, sl], scalar=sd, in1=lng_t[:, sl],
                        op0=OP.mult, op1=OP.mult,
                    )
                    nc.vector.tensor_add(xo[:, sl], xo[:, sl], lnb_t[:, sl])
                    oengs[ch].dma_start(
                        out=out_d.ap()[lb * 128:(lb + 1) * 128, sl],
                        in_=xo[:, sl])

        # row-block 1's quarters accumulate in fresh rotating PSUM banks (so
        # the PE never waits on row-block 0's DVE drain), while row-block 0
        # drains and LayerNorms lazily on DVE/ACT underneath.
        p1s = []
        for nq in range(4):
            p1 = ps.tile([128, 512], F32, tag="ps", name=f"o1_{nq}")
            for j2 in range(8):
                nc.tensor.matmul(
                    p1,
                    attnT_t[:, 2 * j2:2 * j2 + 2, 128:L],
                    wo_panels[nq][:, 2 * j2:2 * j2 + 2, :],
                    start=(j2 == 0),
                    stop=(j2 == 7),
                    perf_mode=DR,
                )
            p1s.append(p1)
            drain_quarter(0, nq, outp[:, nq, :])
        drain_quarter(1, 0, p1s[0])
        drain_quarter(1, 1, p1s[1])
        emit_ln(0)
        drain_quarter(1, 2, p1s[2])
        drain_quarter(1, 3, p1s[3])
        emit_ln(1)


def _pow2scale(w):
    """Power-of-two scale that normalizes w's std to ~1 for fp8 casting."""
    s = float(w.std())
    if not np.isfinite(s) or s == 0.0:
        return 1.0
    return float(2.0 ** np.round(np.log2(1.0 / s)))


def _host_prep(query, key, value, Wq, Wk, Wv, Wo, bo, ln_g, ln_b, alpha, beta):
    """Host-side: cumulative_state shortcut + layout/dtype marshaling."""
    a, b = float(alpha), float(beta)
    f64 = np.float64
    # mean over (batch, l) of kv[b,h,l,m] = (1/(B*L)) sum_b Ksum[b,h,:].V[b,h,m,:]
    keysum = key.astype(f64).sum(axis=1)                      # [B, D]
    Ksum = (keysum @ Wk.T.astype(f64)).reshape(B, H, DH)      # [B, H, DH]
    WvH = Wv.astype(f64).reshape(H, DH, D)
    wv_eff = np.einsum("hdj,bhd->bhj", WvH, Ksum, optimize=True)      # [B,H,D]
    contrib = np.einsum("bmj,bhj->hm", value.astype(f64), wv_eff, optimize=True)
    mean_kv = contrib / (B * L)                               # [H, DH]
    cs = np.zeros((H, DH), f64)
    c = np.zeros(DH, f64)
    for h in range(H):
        cs[h] = c
        c = b * c + a * mean_kv[h]
    # q_mod = Q*((1-a)*cs + a*kv) = (a*Q)*(kv + (1-a)/a*cs); a is folded
    # into Wq below, and this is cs*(1-a)/a:
    csp = ((1.0 - a) / a * cs if a != 0 else 0.0 * cs).astype(np.float32)
    csp_dev = np.ascontiguousarray(
        csp.reshape(H, 2, 128).transpose(2, 0, 1).reshape(128, H * 2)
    )
    plain_ln = bool(np.all(ln_g == 1.0) and np.all(ln_b == 0.0))

    bf = ml_dtypes.bfloat16
    f8 = ml_dtypes.float8_e4m3

    def xpanel(x):  # [B,L,D] -> [B,128,NB,L] with [p,n,l]=xT[n*128+p,l]
        xt = x.transpose(0, 2, 1).reshape(B, NB, 128, L)
        return np.ascontiguousarray(xt.transpose(0, 2, 1, 3)).astype(f8)

    def wpanel(wT, pw):  # [D,D] -> [D//pw,128,NB,pw] panel-major blocks
        wp = wT.reshape(NB, 128, D // pw, pw)
        return np.ascontiguousarray(wp.transpose(2, 1, 0, 3)).astype(f8)

    qT = xpanel(query)
    kT = xpanel(key)
    vT = xpanel(value)
    wqTf = a * Wq.T
    s_q, s_k, s_v, s_o = (_pow2scale(wqTf), _pow2scale(Wk), _pow2scale(Wv),
                          _pow2scale(Wo))
    wqT = wpanel(s_q * wqTf, 512)
    wkT = wpanel(s_k * Wk.T, 512)
    wvT = wpanel(s_v * Wv.T, 512)
    woT = wpanel(s_o * Wo.T, 512)
    invs = (1.0 / s_k, 1.0 / s_v, 1.0 / s_q, 1.0 / s_o)
    qres = (query + bo[None, None, :]).astype(bf)
    # maskT[i,l] = MSCALE iff i<=l
    maskT = (MSCALE * np.triu(np.ones((L, L), np.float32))).astype(bf)

    in_maps = []
    for c_ in range(B):
        in_maps.append({
            "qT": qT[c_], "kT": kT[c_], "vT": vT[c_],
            "qres": qres[c_],
            "wqT": wqT, "wkT": wkT, "wvT": wvT, "woT": woT,
            "csp": csp_dev, "maskT": maskT,
            "lng": ln_g.astype(np.float32), "lnb": ln_b.astype(np.float32),
        })
    return in_maps, a, invs, plain_ln


def get_nc(alpha: float, invs: tuple, plain_ln: bool = True):
    key = (round(float(alpha), 9), tuple(invs), bool(plain_ln))
    if key not in _cache:
        _cache[key] = _build(float(alpha), tuple(invs), bool(plain_ln))
    return _cache[key]


def kernel(query, key, value, Wq, Wk, Wv, Wo, bo, ln_g, ln_b, alpha, beta,
           _trace=False, _trace_kwargs=None):
    args = [np.asarray(x) for x in
            (query, key, value, Wq, Wk, Wv, Wo, bo, ln_g, ln_b, alpha, beta)]
    in_maps, a, invs, plain_ln = _host_prep(*args)
    nc = get_nc(a, invs, plain_ln)
    res = run_bass_kernel_spmd(
        nc, in_maps, core_ids=list(range(B)),
        trace=_trace, **(_trace_kwargs or {}),
    )
    out = np.stack([res.results[c]["out"] for c in range(B)], axis=0)
    if _trace:
        kernel._last_results = res
    return out
